# revision 89
# baseline (speedup 1.0000x reference)
"""Differential entropy regularization (retrieval_knn) on 8 Trainium2 cores.

loss = -mean_i log( mean_{k in top5} ||xn_i - xn_j(k)|| + eps ),  xn = row-normalized x.

Key algebra used by the kernel:
  * For unit rows, distance(i,j) = sqrt(2 - 2 * dot(xn_i, xn_j)), so only the
    top-5 dot VALUES per row are needed (no index gathers).
  * The self-dot (== 1) is always the strict row max, so taking the top-8
    values per row via the DVE max8 instruction and dropping element 0
    replaces diagonal masking.
  * Scaling row i of the similarity matrix by a positive constant r_i does not
    change which columns are its top-5.  So the stationary (lhsT) operand
    stays UN-normalized (raw rows); only the moving operand (all columns) is
    normalized, and the row scale is divided back out inside the final
    sqrt(2 - 2v) evaluation via a per-partition activation scale.

Sharding: rows are split 1024 per core; every core receives the full x^T
(moving operand, fp32r) so the gram needs no communication.  Reciprocal row
norms are computed per-core from the natural-layout row slice (ACT square
with accumulate), AllGathered (8 x 4KB) to give every core all 8192 column
norms, partition-broadcast by DMA, and multiplied into x^T (DVE for the
early column units on the critical path, GpSimd for the late ones).
Per core: 512 fp32r matmuls (N=512, full PE rate) -> [128, 1024] PSUM tiles
-> DVE max8 straight out of PSUM -> merge -> ACT sqrt/log partials -> host
mean.  DMA issue order is tuned so the norm chain and the first gram unit's
operands land before the bulk of x^T streams in.
"""

import numpy as np


def _ensure_path():
    try:
        import concourse.bass  # noqa: F401
    except ImportError:
        import sys

        for p in ("/opt/trn_rl_repo", "/root/.axon_site/_ro/trn_rl_repo"):
            if p not in sys.path:
                sys.path.insert(0, p)
        import concourse.bass  # noqa: F401


N = 8192  # total rows
D = 512  # feature dim
NCORES = 8
RPC = N // NCORES  # rows per core (1024)
P = 128  # partitions
KC = D // P  # contraction chunks (4)
CW = 512  # matmul moving free dim
U = 1024  # gram unit width (PSUM tile)
NU = N // U  # 8 gram units
RT = RPC // P  # row tiles per core (8)
EPS = 1e-8

_NC_CACHE = {}


def _build_nc():
    """Build the (identical-per-core) Bass program once."""
    import concourse.bass as bass  # noqa: F401
    import concourse.tile as tile
    from concourse import bacc, mybir
    from contextlib import ExitStack

    f32 = mybir.dt.float32
    f32r = mybir.dt.float32r
    AF = mybir.ActivationFunctionType

    nc = bacc.Bacc(trn_type="TRN2", target_bir_lowering=False, debug=False)

    xt_d = nc.dram_tensor("xt", [D, N], f32r, kind="ExternalInput")
    xtr_d = nc.dram_tensor("xtr", [D, RPC], f32r, kind="ExternalInput")
    xr_d = nc.dram_tensor("xr", [RPC, D], f32, kind="ExternalInput")
    ones_d = nc.dram_tensor("onesvec", [P, 1], f32r, kind="ExternalInput")
    out_d = nc.dram_tensor("out", [P, RT], f32, kind="ExternalOutput")
    rn_own_d = nc.dram_tensor("rn_own", [1, RPC], f32)
    rn_all_d = nc.dram_tensor("rn_all", [1, N], f32)

    with ExitStack() as ctx:
        tc = ctx.enter_context(tile.TileContext(nc))
        res = ctx.enter_context(tc.tile_pool(name="res", bufs=1))

        SSR = res.tile([P, RT], f32, name="ssr")
        SROOT = res.tile([P, RT], f32, name="sroot")
        RINV = res.tile([P, RT], f32, name="rinv")
        RM2 = res.tile([P, RT], f32, name="rm2")
        btwo = res.tile([P, 1], f32, name="btwo")
        nc.vector.memset(btwo, 2.0)
        beps = res.tile([P, 1], f32, name="beps")
        nc.vector.memset(beps, EPS)
        # preload the ACT Sqrt table while the first DMAs run
        warm = res.tile([P, 1], f32, name="warm")
        nc.scalar.activation(warm, btwo, AF.Sqrt)

        NLOC = 1  # leading units with locally computed column norms
        RN = [res.tile([P, U], f32, name=f"rn_{u}") for u in range(NU)]
        scratch = ctx.enter_context(tc.tile_pool(name="scratch", bufs=2))
        NSB = U // CW  # 512-wide sub-chunks per unit
        XT = {}
        for u in range(NU):
            for k in range(KC):
                for c in range(NSB):
                    XT[k, u, c] = res.tile([P, CW], f32r, name=f"xt_{k}_{u}_{c}")
        XTR = []
        scratch0 = ctx.enter_context(tc.tile_pool(name="scratch0", bufs=4))
        ones = res.tile([P, 1], f32r, name="ones")
        nc.sync.dma_start(ones, ones_d.ap())
        ones1 = res.tile([1, P], f32r, name="ones1")
        nc.sync.dma_start(ones1, ones_d.ap().rearrange("p o -> o p"))

        # ---- critical-path loads first: xr (norm chain), on the SP queue ---
        XR = []
        for rt in range(RT):
            xrt = scratch0.tile([P, D], f32, tag="xr", name=f"xrt{rt}")
            nc.sync.dma_start(xrt, xr_d.ap()[rt * P : (rt + 1) * P, :])
            XR.append(xrt)

        # ---- own-row norms + AllGather (covers units NLOC..) ---------------
        for rt in range(RT):
            dummy = scratch.tile([P, D], f32, tag="sq")
            nc.scalar.activation(
                dummy, XR[rt], AF.Square, accum_out=SSR[:, rt : rt + 1]
            )
        nc.scalar.activation(SROOT, SSR, AF.Sqrt)
        nc.vector.reciprocal(RINV, SROOT)
        nc.vector.tensor_scalar_mul(RM2, RINV, -2.0)
        # rn_own[rt*P + p] = RINV[p, rt]
        nc.sync.dma_start(rn_own_d.ap().rearrange("o (t p) -> o p t", p=P), RINV)
        nc.gpsimd.collective_compute(
            "AllGather",
            mybir.AluOpType.bypass,
            replica_groups=[list(range(NCORES))],
            ins=[rn_own_d.ap()],
            outs=[rn_all_d.ap()],
        )
        for u in range(NLOC, NU):
            nc.sync.dma_start(
                RN[u], rn_all_d.ap()[:, u * U : (u + 1) * U].to_broadcast((P, U))
            )

        def load_unit(u):
            for k in range(KC):
                for c in range(NSB):
                    nc.scalar.dma_start(
                        XT[k, u, c],
                        xt_d.ap()[
                            k * P : (k + 1) * P,
                            u * U + c * CW : u * U + (c + 1) * CW,
                        ],
                    )

        # local column norms (no AllGather): squares -> ones-matmul column
        # sums -> sqrt -> reciprocal -> PE k=1 broadcast matmul -> in-place
        # normalize.  All on-chip, so the first gram units start early.
        def local_norm(u, psn):
            # fully per-sub: each 512-wide slice's norms, broadcast, and
            # normalize complete independently so gram tiles on sub 0 can
            # start while sub 1's chain is still running
            for sub in range(U // CW):
                psl = psn.tile([1, CW], f32, tag="ssps")
                for k in range(KC):
                    sq = scratch.tile([P, CW], f32r, tag="sq")
                    nc.scalar.activation(sq, XT[k, u, sub], AF.Square)
                    nc.tensor.matmul(
                        psl, lhsT=ones, rhs=sq,
                        start=(k == 0), stop=(k == KC - 1),
                    )
                ssb = scratch.tile([1, CW], f32, tag="ssb")
                nc.scalar.activation(ssb, psl, AF.Sqrt)
                rnb = scratch.tile([1, CW], f32r, tag="rnb")
                with nc.allow_low_precision(
                    reason="fp32r is fp32-width; rounding is intended for PE ingest"
                ):
                    nc.vector.reciprocal(rnb, ssb)
                psb = psn.tile([P, CW], f32, tag="bc")
                nc.tensor.matmul(psb, lhsT=ones1, rhs=rnb, start=True, stop=True)
                nc.vector.tensor_copy(RN[u][:, sub * CW : (sub + 1) * CW], psb)
                for k in range(KC):
                    nc.vector.tensor_mul(
                        XT[k, u, sub],
                        XT[k, u, sub],
                        RN[u][:, sub * CW : (sub + 1) * CW],
                    )

        # ---- bulk loads interleaved with the unit-0 local-norm chain -------
        with tc.tile_pool(name="psn", bufs=2, space="PSUM") as psn:
            load_unit(0)
            for k in range(KC):
                t = res.tile([P, RPC], f32r, name=f"xtr_{k}")
                nc.scalar.dma_start(t, xtr_d.ap()[k * P : (k + 1) * P, :])
                XTR.append(t)
            local_norm(0, psn)
        for u in range(NLOC, NU):
            load_unit(u)
        # normalize the gathered units in place (GpSimd, in the shadow of the
        # gram).  Unit NLOC is NOT pre-normalized: its matmuls run on the raw
        # operand (available long before the AllGather lands) and the column
        # scale is applied to the PSUM tile just before max8 instead.
        for u in range(NLOC + 1, NU):
            for k in range(KC):
                for c in range(NSB):
                    nc.gpsimd.tensor_mul(
                        XT[k, u, c],
                        XT[k, u, c],
                        RN[u][:, c * CW : (c + 1) * CW],
                    )

        # ---- gram units + per-row top-8 + loss partials --------------------
        psg = ctx.enter_context(tc.tile_pool(name="psg", bufs=8, space="PSUM"))
        gp = ctx.enter_context(tc.tile_pool(name="gp", bufs=1))
        NSUB = U // CW
        CAND = [
            gp.tile([P, 8 * NU * NSUB], f32, tag=f"cand{rt}", name=f"cand{rt}")
            for rt in range(RT)
        ]
        RHO = res.tile([P, RT], f32, name="rho")
        OUT = res.tile([P, RT], f32, name="outv")
        f5p = ctx.enter_context(tc.tile_pool(name="f5p", bufs=2))
        for u in range(NU):
            # unit 0's tiles sub-major so sub-0 gram work overlaps sub-1's
            # still-running local-norm chain
            order = [(sub, rt) for rt in range(RT) for sub in range(NSUB)]
            for sub, rt in order:
                ps = psg.tile([P, CW], f32, tag="gram")
                for k in range(KC):
                    nc.tensor.matmul(
                        ps,
                        lhsT=XTR[k][:, rt * P : (rt + 1) * P],
                        rhs=XT[k, u, sub],
                        start=(k == 0),
                        stop=(k == KC - 1),
                    )
                if u == NLOC:
                    # deferred column normalize: scale raw dots in PSUM
                    nc.vector.tensor_mul(
                        ps, ps, RN[u][:, sub * CW : (sub + 1) * CW]
                    )
                seg = (u * NSUB + sub) * 8
                nc.vector.max(out=CAND[rt][:, seg : seg + 8], in_=ps)
                if u == NU - 1:
                    # row tile complete: merge + evaluate immediately
                    top8 = f5p.tile([P, 8], f32, tag="top8")
                    nc.vector.max(out=top8, in_=CAND[rt])
                    # f = sqrt(2 - 2 * v / r_i) (values are r_i-scaled)
                    f5 = f5p.tile([P, 5], f32, tag="f5")
                    nc.scalar.activation(
                        f5,
                        top8[:, 1:6],
                        AF.Sqrt,
                        bias=btwo[:, 0:1],
                        scale=RM2[:, rt : rt + 1],
                        accum_out=RHO[:, rt : rt + 1],
                    )
        # out = ln(rho/5 + eps), batched over all row tiles
        nc.scalar.activation(OUT, RHO, AF.Ln, bias=beps[:, 0:1], scale=0.2)
        nc.sync.dma_start(out_d.ap(), OUT)

    nc.compile()
    return nc


def get_nc():
    if "nc" not in _NC_CACHE:
        _ensure_path()
        _NC_CACHE["nc"] = _build_nc()
    return _NC_CACHE["nc"]


def make_in_maps(x):
    x = np.ascontiguousarray(np.asarray(x, dtype=np.float32))
    assert x.shape == (N, D), x.shape
    xt = np.ascontiguousarray(x.T)
    in_maps = []
    for c in range(NCORES):
        in_maps.append(
            {
                "xt": xt,
                "xtr": np.ascontiguousarray(xt[:, c * RPC : (c + 1) * RPC]),
                "xr": np.ascontiguousarray(x[c * RPC : (c + 1) * RPC, :]),
                "onesvec": np.ones((P, 1), dtype=np.float32),
            }
        )
    return in_maps


def combine(results):
    """results: list (per core) of {"out": [P, RT]} -> scalar loss."""
    vals = []
    for c in range(NCORES):
        o = np.asarray(results[c]["out"])  # [P, RT]; row = c*RPC + rt*P + p
        vals.append(o.T.reshape(-1))
    allv = np.concatenate(vals)
    return np.array(-np.mean(allv), dtype=np.float32)


def run(x, **spmd_kwargs):
    _ensure_path()
    from concourse.bass_utils import run_bass_kernel_spmd

    nc = get_nc()
    res = run_bass_kernel_spmd(nc, make_in_maps(x), list(range(NCORES)), **spmd_kwargs)
    return combine(res.results), res


def kernel(x):
    loss, _ = run(x)
    return loss


# revision 90
# speedup vs baseline: 1.3837x; 1.3837x over previous
"""Differential entropy regularization (retrieval_knn) on 8 Trainium2 cores.

loss = -mean_i log( mean_{k in top5} ||xn_i - xn_j(k)|| + eps ),  xn = row-normalized x.

Key algebra used by the kernel:
  * For unit rows, distance(i,j) = sqrt(2 - 2 * dot(xn_i, xn_j)), so only the
    top-5 dot VALUES per row are needed (no index gathers).
  * The self-dot (== 1) is always the strict row max, so taking the top-8
    values per row via the DVE max8 instruction and dropping element 0
    replaces diagonal masking.
  * Scaling row i of the similarity matrix by a positive constant r_i does not
    change which columns are its top-5.  So the stationary (lhsT) operand
    stays UN-normalized (raw rows); only the moving operand (all columns) is
    normalized, and the row scale is divided back out inside the final
    sqrt(2 - 2v) evaluation via a per-partition activation scale.

Sharding: rows are split 1024 per core; every core receives the full x^T
(moving operand, fp32r) so the gram needs no communication.  Reciprocal row
norms are computed per-core from the natural-layout row slice (ACT square
with accumulate), AllGathered (8 x 4KB) to give every core all 8192 column
norms, partition-broadcast by DMA, and multiplied into x^T (DVE for the
early column units on the critical path, GpSimd for the late ones).
Per core: 512 fp32r matmuls (N=512, full PE rate) -> [128, 1024] PSUM tiles
-> DVE max8 straight out of PSUM -> merge -> ACT sqrt/log partials -> host
mean.  DMA issue order is tuned so the norm chain and the first gram unit's
operands land before the bulk of x^T streams in.
"""

import numpy as np


def _ensure_path():
    try:
        import concourse.bass  # noqa: F401
    except ImportError:
        import sys

        for p in ("/opt/trn_rl_repo", "/root/.axon_site/_ro/trn_rl_repo"):
            if p not in sys.path:
                sys.path.insert(0, p)
        import concourse.bass  # noqa: F401


N = 8192  # total rows
D = 512  # feature dim
NCORES = 8
RPC = N // NCORES  # rows per core (1024)
P = 128  # partitions
KC = D // P  # contraction chunks (4)
CW = 512  # matmul moving free dim
U = 1024  # gram unit width (PSUM tile)
NU = N // U  # 8 gram units
RT = RPC // P  # row tiles per core (8)
EPS = 1e-8

_NC_CACHE = {}


def _build_nc():
    """Build the (identical-per-core) Bass program once."""
    import concourse.bass as bass  # noqa: F401
    import concourse.tile as tile
    from concourse import bacc, mybir
    from contextlib import ExitStack

    f32 = mybir.dt.float32
    f32r = mybir.dt.float32r
    AF = mybir.ActivationFunctionType

    nc = bacc.Bacc(trn_type="TRN2", target_bir_lowering=False, debug=False)

    xt_d = nc.dram_tensor("xt", [D, N], f32r, kind="ExternalInput")
    xtr_d = nc.dram_tensor("xtr", [D, RPC], f32r, kind="ExternalInput")
    xr_d = nc.dram_tensor("xr", [RPC, D], f32, kind="ExternalInput")
    ones_d = nc.dram_tensor("onesvec", [P, 1], f32r, kind="ExternalInput")
    out_d = nc.dram_tensor("out", [P, RT], f32, kind="ExternalOutput")
    rn_own_d = nc.dram_tensor("rn_own", [1, RPC], f32)
    rn_all_d = nc.dram_tensor("rn_all", [1, N], f32)

    with ExitStack() as ctx:
        tc = ctx.enter_context(tile.TileContext(nc))
        res = ctx.enter_context(tc.tile_pool(name="res", bufs=1))

        SSR = res.tile([P, RT], f32, name="ssr")
        SROOT = res.tile([P, RT], f32, name="sroot")
        RINV = res.tile([P, RT], f32, name="rinv")
        RM2 = res.tile([P, RT], f32, name="rm2")
        btwo = res.tile([P, 1], f32, name="btwo")
        nc.vector.memset(btwo, 2.0)
        beps = res.tile([P, 1], f32, name="beps")
        nc.vector.memset(beps, EPS)
        # preload the ACT Sqrt table while the first DMAs run
        warm = res.tile([P, 1], f32, name="warm")
        nc.scalar.activation(warm, btwo, AF.Sqrt)

        NLOC = 1  # leading units with locally computed column norms
        RN = [res.tile([P, U], f32, name=f"rn_{u}") for u in range(NU)]
        scratch = ctx.enter_context(tc.tile_pool(name="scratch", bufs=2))
        NSB = U // CW  # 512-wide sub-chunks per unit
        XT = {}
        for u in range(NU):
            for k in range(KC):
                for c in range(NSB):
                    XT[k, u, c] = res.tile([P, CW], f32r, name=f"xt_{k}_{u}_{c}")
        XTR = []
        scratch0 = ctx.enter_context(tc.tile_pool(name="scratch0", bufs=4))
        ones = res.tile([P, 1], f32r, name="ones")
        nc.sync.dma_start(ones, ones_d.ap())
        ones1 = res.tile([1, P], f32r, name="ones1")
        nc.sync.dma_start(ones1, ones_d.ap().rearrange("p o -> o p"))

        # ---- critical-path loads first: xr (norm chain), on the SP queue ---
        XR = []
        for rt in range(RT):
            xrt = scratch0.tile([P, D], f32, tag="xr", name=f"xrt{rt}")
            nc.sync.dma_start(xrt, xr_d.ap()[rt * P : (rt + 1) * P, :])
            XR.append(xrt)

        # ---- own-row norms + AllGather (covers units NLOC..) ---------------
        for rt in range(RT):
            dummy = scratch.tile([P, D], f32, tag="sq")
            nc.scalar.activation(
                dummy, XR[rt], AF.Square, accum_out=SSR[:, rt : rt + 1]
            )
        nc.scalar.activation(SROOT, SSR, AF.Sqrt)
        nc.vector.reciprocal(RINV, SROOT)
        nc.vector.tensor_scalar_mul(RM2, RINV, -2.0)
        # rn_own[rt*P + p] = RINV[p, rt]
        nc.sync.dma_start(rn_own_d.ap().rearrange("o (t p) -> o p t", p=P), RINV)
        nc.gpsimd.collective_compute(
            "AllGather",
            mybir.AluOpType.bypass,
            replica_groups=[list(range(NCORES))],
            ins=[rn_own_d.ap()],
            outs=[rn_all_d.ap()],
        )
        for u in range(NLOC, NU):
            nc.sync.dma_start(
                RN[u], rn_all_d.ap()[:, u * U : (u + 1) * U].to_broadcast((P, U))
            )

        def load_unit(u):
            for k in range(KC):
                for c in range(NSB):
                    nc.scalar.dma_start(
                        XT[k, u, c],
                        xt_d.ap()[
                            k * P : (k + 1) * P,
                            u * U + c * CW : u * U + (c + 1) * CW,
                        ],
                    )

        # local column norms (no AllGather): squares -> ones-matmul column
        # sums -> sqrt -> reciprocal -> PE k=1 broadcast matmul -> in-place
        # normalize.  All on-chip, so the first gram units start early.
        def local_norm(u, psn):
            # fully per-sub: each 512-wide slice's norms, broadcast, and
            # normalize complete independently so gram tiles on sub 0 can
            # start while sub 1's chain is still running
            for sub in range(U // CW):
                psl = psn.tile([1, CW], f32, tag="ssps")
                for k in range(KC):
                    sq = scratch.tile([P, CW], f32r, tag="sq")
                    nc.scalar.activation(sq, XT[k, u, sub], AF.Square)
                    nc.tensor.matmul(
                        psl, lhsT=ones, rhs=sq,
                        start=(k == 0), stop=(k == KC - 1),
                    )
                ssb = scratch.tile([1, CW], f32, tag="ssb")
                nc.scalar.activation(ssb, psl, AF.Sqrt)
                rnb = scratch.tile([1, CW], f32r, tag="rnb")
                with nc.allow_low_precision(
                    reason="fp32r is fp32-width; rounding is intended for PE ingest"
                ):
                    nc.vector.reciprocal(rnb, ssb)
                psb = psn.tile([P, CW], f32, tag="bc")
                nc.tensor.matmul(psb, lhsT=ones1, rhs=rnb, start=True, stop=True)
                nc.vector.tensor_copy(RN[u][:, sub * CW : (sub + 1) * CW], psb)
                for k in range(KC):
                    nc.vector.tensor_mul(
                        XT[k, u, sub],
                        XT[k, u, sub],
                        RN[u][:, sub * CW : (sub + 1) * CW],
                    )

        # ---- bulk loads interleaved with the unit-0 local-norm chain -------
        with tc.tile_pool(name="psn", bufs=2, space="PSUM") as psn:
            load_unit(0)
            for k in range(KC):
                t = res.tile([P, RPC], f32r, name=f"xtr_{k}")
                nc.scalar.dma_start(t, xtr_d.ap()[k * P : (k + 1) * P, :])
                XTR.append(t)
            local_norm(0, psn)
        for u in range(NLOC, NU):
            load_unit(u)
        # normalize the gathered units in place (GpSimd, in the shadow of the
        # gram).  Unit NLOC is NOT pre-normalized: its matmuls run on the raw
        # operand (available long before the AllGather lands) and the column
        # scale is applied to the PSUM tile just before max8 instead.
        for u in range(NLOC + 1, NU):
            for k in range(KC):
                for c in range(NSB):
                    nc.gpsimd.tensor_mul(
                        XT[k, u, c],
                        XT[k, u, c],
                        RN[u][:, c * CW : (c + 1) * CW],
                    )

        # ---- gram units + per-row top-8 + loss partials --------------------
        psg = ctx.enter_context(tc.tile_pool(name="psg", bufs=8, space="PSUM"))
        gp = ctx.enter_context(tc.tile_pool(name="gp", bufs=1))
        NSUB = U // CW
        CAND = [
            gp.tile([P, 8 * NU * NSUB], f32, tag=f"cand{rt}", name=f"cand{rt}")
            for rt in range(RT)
        ]
        RHO = res.tile([P, RT], f32, name="rho")
        OUT = res.tile([P, RT], f32, name="outv")
        f5p = ctx.enter_context(tc.tile_pool(name="f5p", bufs=2))
        for u in range(NU):
            # unit 0's tiles sub-major so sub-0 gram work overlaps sub-1's
            # still-running local-norm chain
            order = [(sub, rt) for rt in range(RT) for sub in range(NSUB)]
            for sub, rt in order:
                ps = psg.tile([P, CW], f32, tag="gram")
                for k in range(KC):
                    nc.tensor.matmul(
                        ps,
                        lhsT=XTR[k][:, rt * P : (rt + 1) * P],
                        rhs=XT[k, u, sub],
                        start=(k == 0),
                        stop=(k == KC - 1),
                    )
                if u == NLOC:
                    # deferred column normalize: scale raw dots in PSUM
                    nc.vector.tensor_mul(
                        ps, ps, RN[u][:, sub * CW : (sub + 1) * CW]
                    )
                seg = (u * NSUB + sub) * 8
                nc.vector.max(out=CAND[rt][:, seg : seg + 8], in_=ps)
                if u == NU - 1 and sub == NSUB - 1:
                    # row tile complete: merge + evaluate immediately
                    top8 = f5p.tile([P, 8], f32, tag="top8")
                    nc.vector.max(out=top8, in_=CAND[rt])
                    # f = sqrt(2 - 2 * v / r_i) (values are r_i-scaled)
                    f5 = f5p.tile([P, 5], f32, tag="f5")
                    nc.scalar.activation(
                        f5,
                        top8[:, 1:6],
                        AF.Sqrt,
                        bias=btwo[:, 0:1],
                        scale=RM2[:, rt : rt + 1],
                        accum_out=RHO[:, rt : rt + 1],
                    )
        # out = ln(rho/5 + eps), batched over all row tiles
        nc.scalar.activation(OUT, RHO, AF.Ln, bias=beps[:, 0:1], scale=0.2)
        nc.sync.dma_start(out_d.ap(), OUT)

    nc.compile()
    return nc


def get_nc():
    if "nc" not in _NC_CACHE:
        _ensure_path()
        _NC_CACHE["nc"] = _build_nc()
    return _NC_CACHE["nc"]


def make_in_maps(x):
    x = np.ascontiguousarray(np.asarray(x, dtype=np.float32))
    assert x.shape == (N, D), x.shape
    xt = np.ascontiguousarray(x.T)
    in_maps = []
    for c in range(NCORES):
        in_maps.append(
            {
                "xt": xt,
                "xtr": np.ascontiguousarray(xt[:, c * RPC : (c + 1) * RPC]),
                "xr": np.ascontiguousarray(x[c * RPC : (c + 1) * RPC, :]),
                "onesvec": np.ones((P, 1), dtype=np.float32),
            }
        )
    return in_maps


def combine(results):
    """results: list (per core) of {"out": [P, RT]} -> scalar loss."""
    vals = []
    for c in range(NCORES):
        o = np.asarray(results[c]["out"])  # [P, RT]; row = c*RPC + rt*P + p
        vals.append(o.T.reshape(-1))
    allv = np.concatenate(vals)
    return np.array(-np.mean(allv), dtype=np.float32)


def run(x, **spmd_kwargs):
    _ensure_path()
    from concourse.bass_utils import run_bass_kernel_spmd

    nc = get_nc()
    res = run_bass_kernel_spmd(nc, make_in_maps(x), list(range(NCORES)), **spmd_kwargs)
    return combine(res.results), res


def kernel(x):
    loss, _ = run(x)
    return loss
